# revision 42
# baseline (speedup 1.0000x reference)
"""Trainium2 Bass kernel for multi-head attention (B=2, S=2048, D=1024, H=16, HD=64).

Sharding: tensor-parallel over heads. Each of the 8 cores owns 2 heads
(core c -> heads 2c, 2c+1) and computes:
  - q^T, k^T projections for its heads (layout [head_dim*2, seq]); the two
    heads live on partitions 0:64 / 64:128 and are emitted adjacently per
    key tile so their K=64 score matmuls run CONCURRENTLY on the PE via
    row-group tiling (~4ns start skew)
  - v^T projection + one PE transpose per s-tile into [1 | pad63 | v(64)]
    blocks, so the attended matmul's stationary operand carries a ones
    column: the attended matmul then emits the softmax denominators on
    PSUM partition 0 for free
  - per key-tile jt: scores^T for BOTH heads into one [128,1024] PSUM tile
    (A cols 0:512, B cols 512:1024) -> ONE exp ACTIVATE per jt (FD=1024,
    1/sqrt(64) scale fused), double-buffered so scores(jt+2) overlap exp(jt)
  - attended^T accumulated per head, lagging the exp stream by LAG key
    tiles (6 for the first ib, whose filler load is heaviest); each ib's
    output projection is woven into the NEXT ib's early slots (after its
    normalize chain has finished) so the PE FIFO does not stall at ib
    boundaries
  - normalization via reciprocal_approx_fast on the PSUM denominator row +
    GpSimd partition broadcast; both heads' attT written directly by DVE
  - partial output projection out_c = attended_c @ out_w[:, heads_c]^T,
    stored as bf16 (host sums partials in f32 and adds the bias)

DMA: weight loads ride the Activation HWDGE queue (dependency-free, so they
never stall the exp stream behind them in the ScalarE FIFO); x chunks and
output stores ride the SP queue (the last ib's stores split across both).
Prerequisite projections and v transposes are woven into per-(b,ib,slot)
filler slots, emitted AFTER that slot's score+exp, to balance PE load
against the ScalarE exp stream (~1.11us/jt).
"""

import numpy as np
import ml_dtypes

import concourse.bacc as bacc
import concourse.tile as tile
import concourse.mybir as mybir
from concourse.bass_utils import run_bass_kernel_spmd
from concourse.masks import make_identity

B, S, D = 2, 2048, 1024
H, HD = 16, 64
FEA = H * HD  # 1024
NCORES = 8
BS = B * S  # 4096

DT_TILES = 8      # 1024 contraction dim / 128
JT = 16           # key tiles of 128 per batch
IB = 4            # query blocks of 512 per batch
LAG = 2           # attended trails the exp stream by this many key tiles
VW = 256          # v storage width per s-tile: [1|pad63|vA(64) | 1|pad63|vB(64)]

BF16 = mybir.dt.bfloat16
F32 = mybir.dt.float32
AF = mybir.ActivationFunctionType
ALU = mybir.AluOpType

_NC_CACHE = {}


def _emit(tc, xT, wqk, wv, wo, out):
    nc = tc.nc
    with (
        tc.tile_pool(name="consts", bufs=1) as consts,
        tc.tile_pool(name="stp", bufs=8) as stp,
        tc.tile_pool(name="small", bufs=4) as small,
        tc.tile_pool(name="outsb", bufs=4) as outsb,
        tc.tile_pool(name="ps_sc", bufs=2, space="PSUM") as ps_sc,
        tc.tile_pool(name="ps_att", bufs=2, space="PSUM") as ps_att,
        tc.tile_pool(name="ps_work", bufs=2, space="PSUM") as ps_work,
    ):
        xts = [consts.tile([128, BS], BF16, name=f"xt{i}", tag=f"xt{i}") for i in range(DT_TILES)]
        wqk_sb = consts.tile([128, DT_TILES * 256], BF16, tag="wqk")
        wv_sb = consts.tile([128, DT_TILES * 128], BF16, tag="wv")
        wos = consts.tile([128, D], BF16, tag="wo")
        # per-batch activations
        qTs = [consts.tile([128, S], BF16, name=f"qT{b}", tag=f"qT{b}") for b in range(B)]
        kTs = [consts.tile([128, S], BF16, name=f"kT{b}", tag=f"kT{b}") for b in range(B)]
        vsbs = [consts.tile([128, 16 * VW], BF16, name=f"v{b}", tag=f"v{b}") for b in range(B)]
        attTs = [consts.tile([128, S], BF16, name=f"attT{b}", tag=f"attT{b}") for b in range(B)]
        vTs = [consts.tile([128, S], BF16, name=f"vT{b}", tag=f"vT{b}") for b in range(B)]
        ident = consts.tile([128, 128], BF16, tag="ident")

        # Pre-load the exp activation table while input DMAs are in flight.
        warm_in = consts.tile([1, 16], F32, tag="warm_in")
        warm_out = consts.tile([1, 16], F32, tag="warm_out")
        nc.vector.memset(warm_in, 0.0)
        nc.scalar.activation(out=warm_out, in_=warm_in, func=AF.Exp)

        # Weights ride the Activation HWDGE queue: they are dependency-free
        # (never wait mid-FIFO, so the exps queued behind them on ScalarE
        # can't be stalled) and overlap the x loads on the SP queue.
        nc.scalar.dma_start(out=wqk_sb, in_=wqk[:, :, :])
        nc.scalar.dma_start(out=wv_sb, in_=wv[:, :, :])
        nc.scalar.dma_start(out=wos, in_=wo[:, :])
        # x^T: batch 0's chunks in 512-col tiles (first-needed first, fine
        # completion granularity for the prologue/score stream); batch 1's
        # (not consumed until ~75us) in one 2048-col DMA per dt tile — 4KB
        # contiguous descriptors at full path rate and 24 fewer triggers
        # occupying the SP engine before the output stores.
        for cb in range(4):
            for dt in range(DT_TILES):
                nc.sync.dma_start(
                    out=xts[dt][:, cb * 512:(cb + 1) * 512],
                    in_=xT[dt * 128:(dt + 1) * 128, cb * 512:(cb + 1) * 512],
                )
        for dt in range(DT_TILES):
            nc.sync.dma_start(
                out=xts[dt][:, 2048:4096],
                in_=xT[dt * 128:(dt + 1) * 128, 2048:4096],
            )
        def _emit_proj_nb(b, nb, half, dst):
            # one 512-col block of a q^T / k^T projection
            scol = nb * 512
            ps = ps_work.tile([128, 512], F32, name="psw", tag="work")
            for dt in range(DT_TILES):
                nc.tensor.matmul(
                    ps,
                    lhsT=wqk_sb[:, dt * 256 + half * 128: dt * 256 + (half + 1) * 128],
                    rhs=xts[dt][:, b * S + scol: b * S + scol + 512],
                    start=(dt == 0),
                    stop=(dt == DT_TILES - 1),
                )
            nc.vector.tensor_copy(out=dst[:, scol:scol + 512], in_=ps)

        def emit_q(b, nb):
            _emit_proj_nb(b, nb, 0, qTs[b])

        def emit_k(b, nb):
            _emit_proj_nb(b, nb, 1, kTs[b])

        def emit_vT(b, nb):
            # v^T [f=128, s] projection block (s-tiles transposed separately)
            scol = nb * 512
            ps = ps_work.tile([128, 512], F32, name="psw", tag="work")
            for dt in range(DT_TILES):
                nc.tensor.matmul(
                    ps,
                    lhsT=wv_sb[:, dt * 128:(dt + 1) * 128],
                    rhs=xts[dt][:, b * S + scol: b * S + scol + 512],
                    start=(dt == 0),
                    stop=(dt == DT_TILES - 1),
                )
            nc.vector.tensor_copy(out=vTs[b][:, scol:scol + 512], in_=ps)

        def emit_v(b, st):
            # One PE transpose turns v^T's [f=128, s-tile] block into natural
            # [s, f] order, then a strided DVE copy lands it as
            # [1|pad63|vA(64) | 1|pad63|vB(64)]. Ones column first =>
            # softmax denominators land on PSUM partition 0.
            ps = ps_work.tile([128, 128], BF16, name="psv", tag="work")
            nc.tensor.transpose(ps, vTs[b][:, st * 128:(st + 1) * 128], ident)
            vsrc = ps.rearrange("p (two c) -> p two c", two=2)
            vdst = vsbs[b][:, st * VW: st * VW + VW].rearrange(
                "p (two c) -> p two c", two=2
            )[:, :, 64:128]
            nc.vector.tensor_copy(out=vdst, in_=vsrc)

        def normalize(b, ib, h, att_ps):
            # Row 0 of att_ps = softmax denominators, rows 64:128 = attended^T.
            icol = ib * 512
            rrow = small.tile([1, 512], F32, name="rrow", tag="rrow")
            nc.vector.reciprocal_approx_fast(out=rrow, in_=att_ps[0:1, :])
            rb = small.tile([128, 512], F32, name="rb", tag="rb")
            nc.gpsimd.partition_broadcast(rb, rrow)
            nc.vector.tensor_tensor(
                out=attTs[b][h * 64:(h + 1) * 64, icol:icol + 512],
                in0=att_ps[64:128, :],
                in1=rb[64:128, :],
                op=ALU.mult,
            )

        def emit_outproj(b, ib, ks, last=False):
            # output rows for s-tiles `ks` of block ib; the last ib's stores
            # split across both DMA queues (no exps left to stall on ScalarE)
            for k in ks:
                row = b * 16 + ib * 4 + k
                for db in range(2):
                    ps = ps_work.tile([128, 512], F32, name="psw", tag="work")
                    nc.tensor.matmul(
                        ps,
                        lhsT=attTs[b][:, (ib * 4 + k) * 128:(ib * 4 + k + 1) * 128],
                        rhs=wos[:, db * 512:(db + 1) * 512],
                        start=True,
                        stop=True,
                    )
                    osb = outsb.tile([128, 512], BF16, name="osb", tag="osb")
                    nc.vector.tensor_copy(out=osb, in_=ps)
                    eng = nc.scalar if (last and db == 1) else nc.sync
                    eng.dma_start(
                        out=out[row * 128:(row + 1) * 128, db * 512:(db + 1) * 512],
                        in_=osb,
                    )

        # Prerequisite work woven into per-(b,ib,slot) filler slots (emitted
        # AFTER that slot's score+exp so they never delay the exp stream):
        # k blocks before the score matmuls that read them, vT blocks + v
        # transposes before the attended matmuls that read the covered
        # s-tiles (v(b,st) is consumed at slot st+LAG), q blocks before
        # their ib, the previous ib's outproj at slots 2/5 (after its
        # normalize chain has had time to finish — avoids stalling the PE
        # FIFO at the ib boundary).
        FILL = {
            (0, 0): {0: (("vT", 0, 0),), 1: (("v", 0, 0), ("v", 0, 1)),
                     2: (("v", 0, 2), ("v", 0, 3)), 3: (("k", 0, 1),),
                     4: (("vT", 0, 1),), 5: (("v", 0, 4), ("v", 0, 5)),
                     6: (("v", 0, 6), ("v", 0, 7)), 7: (("k", 0, 2),),
                     8: (("vT", 0, 2),), 9: (("v", 0, 8), ("v", 0, 9)),
                     10: (("v", 0, 10), ("v", 0, 11)), 11: (("k", 0, 3),),
                     12: (("vT", 0, 3),), 13: (("v", 0, 12), ("v", 0, 13), ("q", 0, 1)),
                     14: (("v", 0, 14), ("v", 0, 15))},
            (0, 1): {2: (("k", 1, 0),), 8: (("q", 0, 2),)},
            (0, 2): {0: (("q", 1, 0),), 4: (("k", 1, 1),), 8: (("q", 0, 3),),
                     12: (("vT", 1, 0),), 13: (("v", 1, 0),), 14: (("v", 1, 1),)},
            (0, 3): {0: (("q", 1, 1),), 2: (("v", 1, 2),), 4: (("k", 1, 2),),
                     6: (("v", 1, 3),), 8: (("k", 1, 3),), 10: (("vT", 1, 1),),
                     12: (("v", 1, 4),), 13: (("v", 1, 5),), 14: (("v", 1, 6),)},
            (1, 0): {0: (("vT", 1, 2),), 2: (("v", 1, 7),), 3: (("v", 1, 8),),
                     5: (("q", 1, 2),), 6: (("vT", 1, 3),), 8: (("v", 1, 9),),
                     9: (("v", 1, 10),), 10: (("v", 1, 11),), 11: (("v", 1, 12),),
                     12: (("v", 1, 13),), 13: (("v", 1, 14),), 14: (("v", 1, 15),)},
            (1, 1): {4: (("q", 1, 3),)},
            (1, 2): {},
            (1, 3): {},
        }
        EMITTERS = {"q": emit_q, "k": emit_k, "vT": emit_vT, "v": emit_v}

        def emit_attention(b, ib, lag):
            icol = ib * 512
            sts = {}
            attA = attB = None
            prev = (b, ib - 1) if ib else (b - 1, IB - 1)
            for slot in range(JT + lag):
                if slot < JT:
                    jt = slot
                    sc = ps_sc.tile([128, 1024], F32, name="sc", tag="sc")
                    for h in range(2):
                        hsl = slice(h * 64, (h + 1) * 64)
                        nc.tensor.matmul(
                            sc[:, h * 512:(h + 1) * 512],
                            lhsT=kTs[b][hsl, jt * 128:(jt + 1) * 128],
                            rhs=qTs[b][hsl, icol:icol + 512],
                            start=True,
                            stop=True,
                        )
                    st = stp.tile([128, 1024], BF16, name="st", tag="st")
                    nc.scalar.activation(out=st, in_=sc, func=AF.Exp, scale=0.125)
                    sts[jt] = st
                for kind, fb, fnb in FILL[(b, ib)].get(slot, ()):
                    EMITTERS[kind](fb, fnb)
                if prev[0] >= 0 and slot in (2, 5):
                    emit_outproj(prev[0], prev[1], (0, 1) if slot == 2 else (2, 3))
                if slot >= lag:
                    jt = slot - lag
                    st = sts.pop(jt)
                    if attA is None:
                        attA = ps_att.tile([128, 512], F32, name="attA", tag="att")
                        attB = ps_att.tile([128, 512], F32, name="attB", tag="att")
                    for h, att in ((0, attA), (1, attB)):
                        nc.tensor.matmul(
                            att,
                            lhsT=vsbs[b][:, jt * VW + h * 128: jt * VW + h * 128 + 128],
                            rhs=st[:, h * 512:(h + 1) * 512],
                            start=(jt == 0),
                            stop=(jt == JT - 1),
                        )
            normalize(b, ib, 0, attA)
            normalize(b, ib, 1, attB)

        # Prologue projections: the minimum to start (0,0)'s score stream.
        # q and k are dt-interleaved so both consume each wqk/x arrival
        # immediately (k's matmuls would otherwise wait for all of q's).
        qps = ps_work.tile([128, 512], F32, name="psw", tag="work")
        kps = ps_work.tile([128, 512], F32, name="psw", tag="work")
        for dt in range(DT_TILES):
            for half, ps in ((0, qps), (1, kps)):
                nc.tensor.matmul(
                    ps,
                    lhsT=wqk_sb[:, dt * 256 + half * 128: dt * 256 + (half + 1) * 128],
                    rhs=xts[dt][:, 0:512],
                    start=(dt == 0),
                    stop=(dt == DT_TILES - 1),
                )
        nc.vector.tensor_copy(out=qTs[0][:, 0:512], in_=qps)
        nc.vector.tensor_copy(out=kTs[0][:, 0:512], in_=kps)
        # The big ones/pad memsets would otherwise sit AHEAD of the critical
        # q/k projection CASTs in the DVE FIFO (3.5us each); their results
        # are not needed until the first v copies of each batch.
        nc.vector.memset(vsbs[0], 1.0)
        make_identity(nc, ident)
        for b in range(B):
            for ib in range(IB):
                if (b, ib) == (0, 1):
                    nc.vector.memset(vsbs[1], 1.0)
                emit_attention(b, ib, 6 if (b, ib) == (0, 0) else LAG)
        emit_outproj(1, 3, (0, 1, 2, 3), last=True)


def build_nc():
    if "nc" in _NC_CACHE:
        return _NC_CACHE["nc"]
    nc = bacc.Bacc("TRN2", debug=False, num_devices=NCORES)
    xT = nc.dram_tensor("xT", [D, BS], BF16, kind="ExternalInput").ap()
    wqk = nc.dram_tensor("wqk", [128, DT_TILES, 256], BF16, kind="ExternalInput").ap()
    wv = nc.dram_tensor("wv", [128, DT_TILES, 128], BF16, kind="ExternalInput").ap()
    wo = nc.dram_tensor("wo", [128, D], BF16, kind="ExternalInput").ap()
    out = nc.dram_tensor("out", [BS, D], BF16, kind="ExternalOutput").ap()
    with tile.TileContext(nc) as tc:
        _emit(tc, xT, wqk, wv, wo, out)
    nc.compile()
    _NC_CACHE["nc"] = nc
    return nc


def make_in_maps(x, qkv_w):
    """Host-side shard + transpose + cast. Returns per-core input dicts
    (without wo, added by caller)."""
    bf = ml_dtypes.bfloat16
    xT = np.ascontiguousarray(x.reshape(BS, D).T).astype(bf)
    maps = []
    for c in range(NCORES):
        wA = qkv_w[c * 384: c * 384 + 192]
        wB = qkv_w[c * 384 + 192: c * 384 + 384]
        wq = np.concatenate([wA[0:64], wB[0:64]], 0)        # [128, D]
        wk = np.concatenate([wA[64:128], wB[64:128]], 0)    # [128, D]
        wv_ = np.concatenate([wA[128:192], wB[128:192]], 0)  # [128, D]
        wqk_c = np.concatenate([wq, wk], 0).T.astype(bf)     # [D, 256]
        wqk_p = np.ascontiguousarray(
            wqk_c.reshape(DT_TILES, 128, 256).transpose(1, 0, 2))
        wv_c = wv_.T.astype(bf)                              # [D, 128]
        wv_p = np.ascontiguousarray(
            wv_c.reshape(DT_TILES, 128, 128).transpose(1, 0, 2))
        maps.append({"xT": xT, "wqk": wqk_p, "wv": wv_p})
    return maps


def kernel(x, qkv_w, out_w, out_b, _run_kwargs=None):
    x = np.asarray(x, dtype=np.float32)
    qkv_w = np.asarray(qkv_w, dtype=np.float32)
    out_w = np.asarray(out_w, dtype=np.float32)
    out_b = np.asarray(out_b, dtype=np.float32)
    bf = ml_dtypes.bfloat16

    nc = build_nc()
    in_maps = make_in_maps(x, qkv_w)
    for c in range(NCORES):
        wo_c = np.ascontiguousarray(
            out_w[:, c * 128:(c + 1) * 128].T).astype(bf)    # [128, D]
        in_maps[c]["wo"] = wo_c

    res = run_bass_kernel_spmd(
        nc, in_maps, list(range(NCORES)), **(_run_kwargs or {})
    )
    total = np.zeros((BS, D), np.float32)
    for c in range(NCORES):
        total += np.asarray(res.results[c]["out"]).astype(np.float32)
    total += out_b[None, :]
    out = total.reshape(B, S, D)
    if _run_kwargs:
        kernel.last_result = res
    return out


# revision 44
# speedup vs baseline: 1.0102x; 1.0102x over previous
"""Trainium2 Bass kernel for multi-head attention (B=2, S=2048, D=1024, H=16, HD=64).

Sharding: tensor-parallel over heads. Each of the 8 cores owns 2 heads
(core c -> heads 2c, 2c+1) and computes:
  - q^T, k^T projections for its heads (layout [head_dim*2, seq]); the two
    heads live on partitions 0:64 / 64:128 and are emitted adjacently per
    key tile so their K=64 score matmuls run CONCURRENTLY on the PE via
    row-group tiling (~4ns start skew)
  - v^T projection + one PE transpose per s-tile into [1 | pad63 | v(64)]
    blocks, so the attended matmul's stationary operand carries a ones
    column: the attended matmul then emits the softmax denominators on
    PSUM partition 0 for free
  - per key-tile jt: scores^T for BOTH heads into one [128,1024] PSUM tile
    (A cols 0:512, B cols 512:1024) -> ONE exp ACTIVATE per jt (FD=1024,
    1/sqrt(64) scale fused), double-buffered so scores(jt+2) overlap exp(jt)
  - attended^T accumulated per head, lagging the exp stream by LAG key
    tiles (6 for the first ib, whose filler load is heaviest); each ib's
    output projection is woven into the NEXT ib's early slots (after its
    normalize chain has finished) so the PE FIFO does not stall at ib
    boundaries
  - normalization via reciprocal_approx_fast on the PSUM denominator row +
    GpSimd partition broadcast; both heads' attT written directly by DVE
  - partial output projection out_c = attended_c @ out_w[:, heads_c]^T,
    stored as bf16 (host sums partials in f32 and adds the bias)

DMA: weight loads ride the Activation HWDGE queue (dependency-free, so they
never stall the exp stream behind them in the ScalarE FIFO); x chunks and
output stores ride the SP queue (the last ib's stores split across both).
Prerequisite projections and v transposes are woven into per-(b,ib,slot)
filler slots, emitted AFTER that slot's score+exp, to balance PE load
against the ScalarE exp stream (~1.11us/jt).
"""

import numpy as np
import ml_dtypes

import concourse.bacc as bacc
import concourse.tile as tile
import concourse.mybir as mybir
from concourse.bass_utils import run_bass_kernel_spmd
from concourse.masks import make_identity

B, S, D = 2, 2048, 1024
H, HD = 16, 64
FEA = H * HD  # 1024
NCORES = 8
BS = B * S  # 4096

DT_TILES = 8      # 1024 contraction dim / 128
JT = 16           # key tiles of 128 per batch
IB = 4            # query blocks of 512 per batch
LAG = 2           # attended trails the exp stream by this many key tiles
VW = 256          # v storage width per s-tile: [1|pad63|vA(64) | 1|pad63|vB(64)]

BF16 = mybir.dt.bfloat16
F32 = mybir.dt.float32
AF = mybir.ActivationFunctionType
ALU = mybir.AluOpType

_NC_CACHE = {}


def _emit(tc, xT, wqk, wv, wo, out):
    nc = tc.nc
    with (
        tc.tile_pool(name="consts", bufs=1) as consts,
        tc.tile_pool(name="stp", bufs=8) as stp,
        tc.tile_pool(name="small", bufs=4) as small,
        tc.tile_pool(name="outsb", bufs=4) as outsb,
        tc.tile_pool(name="ps_sc", bufs=2, space="PSUM") as ps_sc,
        tc.tile_pool(name="ps_att", bufs=2, space="PSUM") as ps_att,
        tc.tile_pool(name="ps_work", bufs=2, space="PSUM") as ps_work,
    ):
        xts = [consts.tile([128, BS], BF16, name=f"xt{i}", tag=f"xt{i}") for i in range(DT_TILES)]
        wqk_sb = consts.tile([128, DT_TILES * 256], BF16, tag="wqk")
        wv_sb = consts.tile([128, DT_TILES * 128], BF16, tag="wv")
        wos = consts.tile([128, D], BF16, tag="wo")
        # per-batch activations
        qTs = [consts.tile([128, S], BF16, name=f"qT{b}", tag=f"qT{b}") for b in range(B)]
        kTs = [consts.tile([128, S], BF16, name=f"kT{b}", tag=f"kT{b}") for b in range(B)]
        vsbs = [consts.tile([128, 16 * VW], BF16, name=f"v{b}", tag=f"v{b}") for b in range(B)]
        attTs = [consts.tile([128, S], BF16, name=f"attT{b}", tag=f"attT{b}") for b in range(B)]
        vTs = [consts.tile([128, S], BF16, name=f"vT{b}", tag=f"vT{b}") for b in range(B)]
        ident = consts.tile([128, 128], BF16, tag="ident")

        # Pre-load the exp activation table while input DMAs are in flight.
        warm_in = consts.tile([1, 16], F32, tag="warm_in")
        warm_out = consts.tile([1, 16], F32, tag="warm_out")
        nc.vector.memset(warm_in, 0.0)
        nc.scalar.activation(out=warm_out, in_=warm_in, func=AF.Exp)

        # Weights ride the Activation HWDGE queue: they are dependency-free
        # (never wait mid-FIFO, so the exps queued behind them on ScalarE
        # can't be stalled) and overlap the x loads on the SP queue.
        nc.scalar.dma_start(out=wqk_sb, in_=wqk[:, :, :])
        nc.scalar.dma_start(out=wv_sb, in_=wv[:, :, :])
        nc.scalar.dma_start(out=wos, in_=wo[:, :])
        # x^T in 512-col chunks, first-needed first, on the SP queue.
        for cb in range(8):
            for dt in range(DT_TILES):
                nc.sync.dma_start(
                    out=xts[dt][:, cb * 512:(cb + 1) * 512],
                    in_=xT[dt * 128:(dt + 1) * 128, cb * 512:(cb + 1) * 512],
                )
        def _emit_proj_nb(b, nb, half, dst):
            # one 512-col block of a q^T / k^T projection
            scol = nb * 512
            ps = ps_work.tile([128, 512], F32, name="psw", tag="work")
            for dt in range(DT_TILES):
                nc.tensor.matmul(
                    ps,
                    lhsT=wqk_sb[:, dt * 256 + half * 128: dt * 256 + (half + 1) * 128],
                    rhs=xts[dt][:, b * S + scol: b * S + scol + 512],
                    start=(dt == 0),
                    stop=(dt == DT_TILES - 1),
                )
            nc.vector.tensor_copy(out=dst[:, scol:scol + 512], in_=ps)

        def emit_q(b, nb):
            _emit_proj_nb(b, nb, 0, qTs[b])

        def emit_k(b, nb):
            _emit_proj_nb(b, nb, 1, kTs[b])

        def emit_vT(b, nb):
            # v^T [f=128, s] projection block (s-tiles transposed separately)
            scol = nb * 512
            ps = ps_work.tile([128, 512], F32, name="psw", tag="work")
            for dt in range(DT_TILES):
                nc.tensor.matmul(
                    ps,
                    lhsT=wv_sb[:, dt * 128:(dt + 1) * 128],
                    rhs=xts[dt][:, b * S + scol: b * S + scol + 512],
                    start=(dt == 0),
                    stop=(dt == DT_TILES - 1),
                )
            nc.vector.tensor_copy(out=vTs[b][:, scol:scol + 512], in_=ps)

        def emit_v(b, st):
            # One PE transpose turns v^T's [f=128, s-tile] block into natural
            # [s, f] order, then a strided DVE copy lands it as
            # [1|pad63|vA(64) | 1|pad63|vB(64)]. Ones column first =>
            # softmax denominators land on PSUM partition 0.
            ps = ps_work.tile([128, 128], BF16, name="psv", tag="work")
            nc.tensor.transpose(ps, vTs[b][:, st * 128:(st + 1) * 128], ident)
            vsrc = ps.rearrange("p (two c) -> p two c", two=2)
            vdst = vsbs[b][:, st * VW: st * VW + VW].rearrange(
                "p (two c) -> p two c", two=2
            )[:, :, 64:128]
            nc.vector.tensor_copy(out=vdst, in_=vsrc)

        def normalize(b, ib, h, att_ps):
            # Row 0 of att_ps = softmax denominators, rows 64:128 = attended^T.
            icol = ib * 512
            rrow = small.tile([1, 512], F32, name="rrow", tag="rrow")
            nc.vector.reciprocal_approx_fast(out=rrow, in_=att_ps[0:1, :])
            rb = small.tile([128, 512], F32, name="rb", tag="rb")
            nc.gpsimd.partition_broadcast(rb, rrow)
            nc.vector.tensor_tensor(
                out=attTs[b][h * 64:(h + 1) * 64, icol:icol + 512],
                in0=att_ps[64:128, :],
                in1=rb[64:128, :],
                op=ALU.mult,
            )

        def emit_outproj(b, ib, ks, last=False):
            # output rows for s-tiles `ks` of block ib; the last ib's stores
            # split across both DMA queues (no exps left to stall on ScalarE)
            for k in ks:
                row = b * 16 + ib * 4 + k
                for db in range(2):
                    ps = ps_work.tile([128, 512], F32, name="psw", tag="work")
                    nc.tensor.matmul(
                        ps,
                        lhsT=attTs[b][:, (ib * 4 + k) * 128:(ib * 4 + k + 1) * 128],
                        rhs=wos[:, db * 512:(db + 1) * 512],
                        start=True,
                        stop=True,
                    )
                    osb = outsb.tile([128, 512], BF16, name="osb", tag="osb")
                    nc.vector.tensor_copy(out=osb, in_=ps)
                    eng = nc.scalar if (last and db == 1) else nc.sync
                    eng.dma_start(
                        out=out[row * 128:(row + 1) * 128, db * 512:(db + 1) * 512],
                        in_=osb,
                    )

        # Prerequisite work woven into per-(b,ib,slot) filler slots (emitted
        # AFTER that slot's score+exp so they never delay the exp stream):
        # k blocks before the score matmuls that read them, vT blocks + v
        # transposes before the attended matmuls that read the covered
        # s-tiles (v(b,st) is consumed at slot st+LAG), q blocks before
        # their ib, the previous ib's outproj at slots 2/5 (after its
        # normalize chain has had time to finish — avoids stalling the PE
        # FIFO at the ib boundary).
        FILL = {
            (0, 0): {2: (("vT", 0, 0),), 3: (("k", 0, 1), ("v", 0, 0)),
                     4: (("vT", 0, 1), ("v", 0, 1)),
                     5: (("v", 0, 2), ("v", 0, 3)),
                     6: (("v", 0, 4), ("v", 0, 5)), 7: (("k", 0, 2), ("v", 0, 6)),
                     8: (("vT", 0, 2), ("v", 0, 7)), 9: (("v", 0, 8), ("v", 0, 9)),
                     10: (("v", 0, 10), ("v", 0, 11)), 11: (("k", 0, 3),),
                     12: (("vT", 0, 3),), 13: (("v", 0, 12), ("v", 0, 13), ("q", 0, 1)),
                     14: (("v", 0, 14), ("v", 0, 15))},
            (0, 1): {2: (("k", 1, 0),), 8: (("q", 0, 2),)},
            (0, 2): {0: (("q", 1, 0),), 4: (("k", 1, 1),), 8: (("q", 0, 3),),
                     12: (("vT", 1, 0),), 13: (("v", 1, 0),), 14: (("v", 1, 1),)},
            (0, 3): {0: (("q", 1, 1),), 2: (("v", 1, 2),), 4: (("k", 1, 2),),
                     6: (("v", 1, 3),), 8: (("k", 1, 3),), 10: (("vT", 1, 1),),
                     12: (("v", 1, 4),), 13: (("v", 1, 5),), 14: (("v", 1, 6),)},
            (1, 0): {0: (("vT", 1, 2),), 2: (("v", 1, 7),), 3: (("v", 1, 8),),
                     5: (("q", 1, 2),), 6: (("vT", 1, 3),), 8: (("v", 1, 9),),
                     9: (("v", 1, 10),), 10: (("v", 1, 11),), 11: (("v", 1, 12),),
                     12: (("v", 1, 13),), 13: (("v", 1, 14),), 14: (("v", 1, 15),)},
            (1, 1): {4: (("q", 1, 3),)},
            (1, 2): {},
            (1, 3): {},
        }
        EMITTERS = {"q": emit_q, "k": emit_k, "vT": emit_vT, "v": emit_v}

        def emit_attention(b, ib, lag):
            icol = ib * 512
            sts = {}
            attA = attB = None
            prev = (b, ib - 1) if ib else (b - 1, IB - 1)
            for slot in range(JT + lag):
                if slot < JT:
                    jt = slot
                    sc = ps_sc.tile([128, 1024], F32, name="sc", tag="sc")
                    for h in range(2):
                        hsl = slice(h * 64, (h + 1) * 64)
                        nc.tensor.matmul(
                            sc[:, h * 512:(h + 1) * 512],
                            lhsT=kTs[b][hsl, jt * 128:(jt + 1) * 128],
                            rhs=qTs[b][hsl, icol:icol + 512],
                            start=True,
                            stop=True,
                        )
                    st = stp.tile([128, 1024], BF16, name="st", tag="st")
                    nc.scalar.activation(out=st, in_=sc, func=AF.Exp, scale=0.125)
                    sts[jt] = st
                for kind, fb, fnb in FILL[(b, ib)].get(slot, ()):
                    EMITTERS[kind](fb, fnb)
                if prev[0] >= 0 and slot in (2, 5):
                    emit_outproj(prev[0], prev[1], (0, 1) if slot == 2 else (2, 3))
                if slot >= lag:
                    jt = slot - lag
                    st = sts.pop(jt)
                    if attA is None:
                        attA = ps_att.tile([128, 512], F32, name="attA", tag="att")
                        attB = ps_att.tile([128, 512], F32, name="attB", tag="att")
                    for h, att in ((0, attA), (1, attB)):
                        nc.tensor.matmul(
                            att,
                            lhsT=vsbs[b][:, jt * VW + h * 128: jt * VW + h * 128 + 128],
                            rhs=st[:, h * 512:(h + 1) * 512],
                            start=(jt == 0),
                            stop=(jt == JT - 1),
                        )
            normalize(b, ib, 0, attA)
            normalize(b, ib, 1, attB)

        # Prologue projections: the minimum to start (0,0)'s score stream.
        # q and k are dt-interleaved so both consume each wqk/x arrival
        # immediately (k's matmuls would otherwise wait for all of q's).
        qps = ps_work.tile([128, 512], F32, name="psw", tag="work")
        kps = ps_work.tile([128, 512], F32, name="psw", tag="work")
        for dt in range(DT_TILES):
            for half, ps in ((0, qps), (1, kps)):
                nc.tensor.matmul(
                    ps,
                    lhsT=wqk_sb[:, dt * 256 + half * 128: dt * 256 + (half + 1) * 128],
                    rhs=xts[dt][:, 0:512],
                    start=(dt == 0),
                    stop=(dt == DT_TILES - 1),
                )
        nc.vector.tensor_copy(out=qTs[0][:, 0:512], in_=qps)
        nc.vector.tensor_copy(out=kTs[0][:, 0:512], in_=kps)
        # The big ones/pad memsets would otherwise sit AHEAD of the critical
        # q/k projection CASTs in the DVE FIFO (3.5us each); their results
        # are not needed until the first v copies of each batch.
        nc.vector.memset(vsbs[0], 1.0)
        make_identity(nc, ident)
        for b in range(B):
            for ib in range(IB):
                if (b, ib) == (0, 1):
                    nc.vector.memset(vsbs[1], 1.0)
                emit_attention(b, ib, 6 if (b, ib) == (0, 0) else LAG)
        emit_outproj(1, 3, (0, 1, 2, 3), last=True)


def build_nc():
    if "nc" in _NC_CACHE:
        return _NC_CACHE["nc"]
    nc = bacc.Bacc("TRN2", debug=False, num_devices=NCORES)
    xT = nc.dram_tensor("xT", [D, BS], BF16, kind="ExternalInput").ap()
    wqk = nc.dram_tensor("wqk", [128, DT_TILES, 256], BF16, kind="ExternalInput").ap()
    wv = nc.dram_tensor("wv", [128, DT_TILES, 128], BF16, kind="ExternalInput").ap()
    wo = nc.dram_tensor("wo", [128, D], BF16, kind="ExternalInput").ap()
    out = nc.dram_tensor("out", [BS, D], BF16, kind="ExternalOutput").ap()
    with tile.TileContext(nc) as tc:
        _emit(tc, xT, wqk, wv, wo, out)
    nc.compile()
    _NC_CACHE["nc"] = nc
    return nc


def make_in_maps(x, qkv_w):
    """Host-side shard + transpose + cast. Returns per-core input dicts
    (without wo, added by caller)."""
    bf = ml_dtypes.bfloat16
    xT = np.ascontiguousarray(x.reshape(BS, D).T).astype(bf)
    maps = []
    for c in range(NCORES):
        wA = qkv_w[c * 384: c * 384 + 192]
        wB = qkv_w[c * 384 + 192: c * 384 + 384]
        wq = np.concatenate([wA[0:64], wB[0:64]], 0)        # [128, D]
        wk = np.concatenate([wA[64:128], wB[64:128]], 0)    # [128, D]
        wv_ = np.concatenate([wA[128:192], wB[128:192]], 0)  # [128, D]
        wqk_c = np.concatenate([wq, wk], 0).T.astype(bf)     # [D, 256]
        wqk_p = np.ascontiguousarray(
            wqk_c.reshape(DT_TILES, 128, 256).transpose(1, 0, 2))
        wv_c = wv_.T.astype(bf)                              # [D, 128]
        wv_p = np.ascontiguousarray(
            wv_c.reshape(DT_TILES, 128, 128).transpose(1, 0, 2))
        maps.append({"xT": xT, "wqk": wqk_p, "wv": wv_p})
    return maps


def kernel(x, qkv_w, out_w, out_b, _run_kwargs=None):
    x = np.asarray(x, dtype=np.float32)
    qkv_w = np.asarray(qkv_w, dtype=np.float32)
    out_w = np.asarray(out_w, dtype=np.float32)
    out_b = np.asarray(out_b, dtype=np.float32)
    bf = ml_dtypes.bfloat16

    nc = build_nc()
    in_maps = make_in_maps(x, qkv_w)
    for c in range(NCORES):
        wo_c = np.ascontiguousarray(
            out_w[:, c * 128:(c + 1) * 128].T).astype(bf)    # [128, D]
        in_maps[c]["wo"] = wo_c

    res = run_bass_kernel_spmd(
        nc, in_maps, list(range(NCORES)), **(_run_kwargs or {})
    )
    total = np.zeros((BS, D), np.float32)
    for c in range(NCORES):
        total += np.asarray(res.results[c]["out"]).astype(np.float32)
    total += out_b[None, :]
    out = total.reshape(B, S, D)
    if _run_kwargs:
        kernel.last_result = res
    return out
